# revision 33
# baseline (speedup 1.0000x reference)
"""Perona-Malik anisotropic diffusion (option 2), 10 iterations, on 8 TRN2 NeuronCores.

Pure data parallel: each core takes 2 of the 16 batch images (= 6 channel-images of
512x512).  Per core, u is double-buffered in SBUF as 6 per-image tiles
[128 rows, 4 bands x 514 cols] bf16 (512 interior cols + 2 zero-pad cols giving
zero-padding semantics for horizontal shifts; 512 rows = exactly 4 x 128 partitions).

Division of labor per band per iteration:
  - TensorEngine produces row-shifted copies pup/pdn [128,514] in PSUM via shift-matrix
    matmuls (band-seam rows come from a one-hot matmul of the band below, and from a
    DMA-maintained partition-0 "shadow" of each band's row 127 for the band above,
    since compute engines cannot address partition 127 directly).
  - ScalarEngine stages pup/pdn to SBUF (ScalarE reads PSUM fast, VectorE reads SBUF
    fast) and applies the final u_{t+1} = DT * upd update (PSUM -> bf16 SBUF).
  - VectorEngine runs ONE fused custom DVE op per direction:
        y_k = (w f d + w b) * (1 - (f d)^2 / (2 kappa^2))^2,   d = shift_k(u) - u
    approximating w * nab / (1 + (nab/kappa)^2) (Taylor in z = (nab/kappa)^2 <= 0.09).
    E/W are batched across all 4 bands of an image in a single FD=2048 op (a no-imm2
    variant whose output scale 42.5 rides the accumulating matmul's lhsT).
  - TensorEngine sums the 8 directional fields plus 7*u into PSUM via (scaled-)identity
    matmuls; 1/DT = 7 is folded into the u term so the final update is a pure scale.

biases/factors are folded into the custom-op scalars at trace time (the kernel is
compiled inside kernel(), cached on the biases/factors bytes).  Measured end-to-end
max rel err vs the exact f32 reference: 3.7e-3 (bf16 state + Taylor approx), well
inside the 2e-2 gate.  HW exec time ~1.39 ms; VectorE (the bottleneck) is >97% busy
at its per-op streaming floor.
"""
import math
import os
import sys

import numpy as np

for _p in ("/root/.axon_site", "/root/.axon_site/_ro/trn_rl_repo", "/opt/trn_rl_repo"):
    if os.path.isdir(_p) and _p not in sys.path:
        sys.path.append(_p)

import concourse.bass as bass
import concourse.tile as tile
from concourse import bacc, mybir
from concourse.bass_utils import run_bass_kernel_spmd

# ---------------- problem constants (hardcoded; kernel.py is self-contained) ---
B, C, H, W = 16, 3, 512, 512
NUM_ITER = 10
DT = 1.0 / 7.0
KAPPA = 30.0
OFFSETS = [(-1, 0), (1, 0), (0, -1), (0, 1), (-1, 1), (1, 1), (1, -1), (-1, -1)]
DIR_W = [1.0, 1.0, 1.0, 1.0, 0.5, 0.5, 0.5, 0.5]

N_CORES = 8
IMGS = (B // N_CORES) * C          # 6 images per core
BANDS_PER_IMG = H // 128           # 4
N_BANDS = IMGS * BANDS_PER_IMG     # 24
WP = W + 2                         # padded width 514

BF16 = mybir.dt.bfloat16
F32 = mybir.dt.float32

# ---------------- custom DVE op: fused diffusion direction ---------------------
from concourse.dve_spec import Spec, Src0, Src1, One, sq, lower
from concourse.dve_ops import (
    OPS,
    DveOp,
    _SUB_OPCODE_FOR_NAME,
    _CUSTOM_DVE_ROW_BASE,
    C0,
    C1,
    C2,
)
from concourse.dve_uop import DveOpSpec


def _pm_ref(in0, in1, s0, s1, imm2):
    d = in0.astype(np.float32) - in1.astype(np.float32)
    m = d * s0
    nt = m + s1
    v = m * imm2
    g = 1.0 - v * v
    return nt * (g * g)


def _register_pm_op():
    name = "PM_DIFFUSE_ANT"
    if name in _SUB_OPCODE_FOR_NAME:
        return next(op for op in OPS if op.name == name)
    _d = Src0 - Src1
    _m = _d * C0
    _nt = _m + C1
    _v = _m * C2
    _g = One - sq(_v)
    spec = Spec(body=_nt * sq(_g), reference=_pm_ref)
    row = _CUSTOM_DVE_ROW_BASE + len(OPS)
    _SUB_OPCODE_FOR_NAME[name] = row
    shas = {}
    for ver in ("v3", "v4"):
        s = DveOpSpec(name=name, opcode=row, uops=lower(spec, ver=ver), rd1_en=True)
        shas[ver] = s.sha(ver)
    op = DveOp(name, spec, subdim=False, uops_sha=shas)
    OPS.append(op)
    return op


PM_OP = _register_pm_op()

# E/W variant: no imm2 slot available (3D in1), so the final scale L=42.5 is
# applied by the accumulating matmul (lhsT = 42.5*I, bf16-exact).
#   out = v*(1 - v^2)^2,  v = (in0-in1)*s0 + s1
# with s0 = w*f/L, s1 = w*b/L and L chosen ~= w*kappa*sqrt(2) so that
# v^2 ~= ((f d + b)/kappa)^2 / 2 (off by (42.4264/42.5)^2 = 0.35%, negligible).
PM_EW_L = 42.5


def _pm_ew_ref(in0, in1, s0, s1, imm2):
    v = (in0.astype(np.float32) - in1.astype(np.float32)) * s0 + s1
    g = 1.0 - v * v
    return v * (g * g)


def _register_pm_ew_op():
    name = "PM_DIFFUSE_EW_ANT"
    if name in _SUB_OPCODE_FOR_NAME:
        return next(op for op in OPS if op.name == name)
    _v = (Src0 - Src1) * C0 + C1
    _g = One - sq(_v)
    spec = Spec(body=_v * sq(_g), reference=_pm_ew_ref)
    row = _CUSTOM_DVE_ROW_BASE + len(OPS)
    _SUB_OPCODE_FOR_NAME[name] = row
    shas = {}
    for ver in ("v3", "v4"):
        sp = DveOpSpec(name=name, opcode=row, uops=lower(spec, ver=ver), rd1_en=True)
        shas[ver] = sp.sha(ver)
    op = DveOp(name, spec, subdim=False, uops_sha=shas)
    OPS.append(op)
    return op


PM_EW_OP = _register_pm_ew_op()


# ---------------- weight matrices for TensorE ---------------------------------
def _weight_mats():
    import ml_dtypes

    S_upT = np.zeros((128, 128), np.float32)   # out[m] = u[m-1]
    S_upT[np.arange(127), np.arange(1, 128)] = 1.0
    S_dnT = np.zeros((128, 128), np.float32)   # out[m] = u[m+1]
    S_dnT[np.arange(1, 128), np.arange(127)] = 1.0
    E_dnT = np.zeros((128, 128), np.float32)   # out[127] = next[0]
    E_dnT[0, 127] = 1.0
    I = np.eye(128, dtype=np.float32)
    I7 = 7.0 * np.eye(128, dtype=np.float32)   # folds 1/DT into the u term
    IL = PM_EW_L * np.eye(128, dtype=np.float32)   # w=1 custom-op output scale
    IL2 = (PM_EW_L / 2) * np.eye(128, dtype=np.float32)  # w=0.5 diagonals
    return np.stack([S_upT, S_dnT, E_dnT, I, I7, IL, IL2]).astype(ml_dtypes.bfloat16)


# ---------------- kernel build -------------------------------------------------
def build_nc(biases: np.ndarray, factors: np.ndarray):
    """Trace the full 10-iteration kernel; biases/factors folded as immediates."""
    biases = np.asarray(biases, np.float32)
    factors = np.asarray(factors, np.float32)

    nc = bacc.Bacc()
    x_d = nc.declare_dram_parameter("x", [IMGS, H, W], F32, isOutput=False)
    w_d = nc.declare_dram_parameter("wmat", [7, 128, 128], BF16, isOutput=False)
    o_d = nc.declare_dram_parameter("out", [IMGS, H, W], F32, isOutput=True)

    with tile.TileContext(nc) as tc:
        from contextlib import ExitStack

        with ExitStack() as ctx:
            upool = ctx.enter_context(tc.tile_pool(name="u", bufs=1))
            wpool = ctx.enter_context(tc.tile_pool(name="w", bufs=1))
            io_pool = ctx.enter_context(tc.tile_pool(name="io", bufs=4))
            y_pool = ctx.enter_context(tc.tile_pool(name="y", bufs=12))  # per-tag bufs below for big tiles
            pup_pool = ctx.enter_context(tc.tile_pool(name="pup", bufs=1, space="PSUM"))
            pdn_pool = ctx.enter_context(tc.tile_pool(name="pdn", bufs=1, space="PSUM"))
            upd_pool = ctx.enter_context(tc.tile_pool(name="upd", bufs=4, space="PSUM"))
            sh_pool = ctx.enter_context(tc.tile_pool(name="sh", bufs=1))
            ps_pool = ctx.enter_context(tc.tile_pool(name="ps", bufs=3))
            import dataclasses as _dc

            def _src_win3(row_ap):
                """[1, W]-row AP -> [1, 3, W] overlapping windows at col offsets 0,1,2."""
                return _dc.replace(row_ap, ap=[row_ap.ap[0], [1, 3], [1, W]])

            def _dst3(row_ap):
                """[1, 3W]-row AP -> [1, 3, W] contiguous split."""
                return _dc.replace(row_ap, ap=[row_ap.ap[0], [W, 3], [1, W]])

            # persistent tiles
            wt = [wpool.tile([128, 128], BF16, tag=f"w{i}", name=f"w{i}") for i in range(7)]
            S_UP, S_DN, E_DN, IDENT, IDENT7, IDENT_L, IDENT_L2 = wt
            uA = [upool.tile([128, BANDS_PER_IMG * WP], BF16, tag=f"uA{i}", name=f"uA{i}") for i in range(IMGS)]
            uB = [upool.tile([128, BANDS_PER_IMG * WP], BF16, tag=f"uB{i}", name=f"uB{i}") for i in range(IMGS)]

            def uv(ub, j):
                img, jb = divmod(j, BANDS_PER_IMG)
                return ub[img][:, jb * WP : (jb + 1) * WP]

            def img_win(ub, img, col, n=W):
                """[128, 4, n] view of image tile: 4 bands at column offset col."""
                base = ub[img][:, col : col + n]
                return _dc.replace(base, ap=[base.ap[0], [WP, BANDS_PER_IMG], [1, n]])

            def y4_split(y_ap):
                """[128, 4*W] tile -> [128, 4, W]."""
                return _dc.replace(y_ap, ap=[y_ap.ap[0], [W, BANDS_PER_IMG], [1, W]])
            # row 127 of each band mirrored at partition 0 (engines cannot read
            # partition 127 directly: partition starts must be quadrant-aligned)
            sh127 = [sh_pool.tile([1, WP], BF16, tag=f"sh{j}", name=f"sh{j}") for j in range(N_BANDS)]

            for i in range(7):
                nc.sync.dma_start(wt[i][:], w_d[i])

            # load input: DMA f32 -> staging, convert to bf16 interior; zero pads
            for j in range(N_BANDS):
                img, jb = divmod(j, BANDS_PER_IMG)
                st = io_pool.tile([128, W], F32, tag="stage_in")
                nc.sync.dma_start(st[:], x_d[img, jb * 128 : (jb + 1) * 128, :])
                for ub in (uA, uB):
                    v = uv(ub, j)
                    nc.gpsimd.memset(v[:, 0:1], 0.0)
                    nc.gpsimd.memset(v[:, WP - 1 : WP], 0.0)
                nc.scalar.copy(uv(uA, j)[:, 1 : W + 1], st[:])
                if jb < BANDS_PER_IMG - 1:
                    nc.sync.dma_start(sh127[j][0:1, :], uv(uA, j)[127:128, :])

            # per-direction constants
            # y = (w f d + w b) * (1 - (f d)^2/(2 kappa^2))^2
            # s0 = w*f[k,c], s1 = w*b[k,c], imm2 = 1/(w*kappa*sqrt(2))
            def consts(k, c):
                wgt = DIR_W[k]
                return (
                    float(wgt * factors[k, c]),
                    float(wgt * biases[k, c]),
                    float(1.0 / (wgt * KAPPA * math.sqrt(2.0))),
                )

            bufs = [uA, uB]
            for t in range(NUM_ITER):
                u_cur = bufs[t % 2]
                u_nxt = bufs[(t + 1) % 2]
                yEW = {}
                upds = {}
                pair = None
                for j in range(N_BANDS):
                    img, jb = divmod(j, BANDS_PER_IMG)
                    ch = img % C
                    if jb == 0:
                        # batched E/W custom ops over the whole image (FD = 4*512)
                        yE = y_pool.tile([128, BANDS_PER_IMG * W], BF16, tag="yE", name="yE", bufs=3)
                        yW = y_pool.tile([128, BANDS_PER_IMG * W], BF16, tag="yW", name="yW", bufs=3)
                        nc.vector._custom_dve(
                            PM_EW_OP, out=y4_split(yE[:, :]),
                            in0=img_win(u_cur, img, 2), in1=img_win(u_cur, img, 1),
                            s0=float(DIR_W[3] * factors[3, ch] / PM_EW_L),
                            s1=float(DIR_W[3] * biases[3, ch] / PM_EW_L),
                        )
                        nc.vector._custom_dve(
                            PM_EW_OP, out=y4_split(yW[:, :]),
                            in0=img_win(u_cur, img, 0), in1=img_win(u_cur, img, 1),
                            s0=float(DIR_W[2] * factors[2, ch] / PM_EW_L),
                            s1=float(DIR_W[2] * biases[2, ch] / PM_EW_L),
                        )
                        yEW[img] = (yE, yW)
                    # --- TensorE: row-shifted copies pup/pdn [128, 514] f32 ---
                    u_band = uv(u_cur, j)
                    pup = pup_pool.tile([128, WP], F32, name="pup")
                    pdn = pdn_pool.tile([128, WP], F32, name="pdn")
                    has_dn = jb < BANDS_PER_IMG - 1
                    for lo, hi in ((0, 512), (512, WP)):
                        nc.tensor.matmul(
                            pup[:, lo:hi], S_UP[:], u_band[:, lo:hi],
                            start=True, stop=True,
                        )
                        nc.tensor.matmul(
                            pdn[:, lo:hi], S_DN[:], u_band[:, lo:hi],
                            start=True, stop=not has_dn,
                        )
                        if has_dn:
                            nc.tensor.matmul(
                                pdn[:, lo:hi], E_DN[:], uv(u_cur, j + 1)[:, lo:hi],
                                start=False, stop=True,
                            )
                    # stage P into SBUF (ScalarE reads PSUM fast; DVE reads SBUF fast).
                    # Bands are staged in PAIRS (jb 0+1, 2+3) side by side so the
                    # N/S custom ops can process two bands in one FD=1024 stream.
                    half = jb % 2
                    if half == 0:
                        pup_w = ps_pool.tile([128, 6 * W], BF16, tag="pup_w", name="pup_w", bufs=2)
                        pdn_w = ps_pool.tile([128, 6 * W], BF16, tag="pdn_w", name="pdn_w", bufs=2)
                        pair = (pup_w, pdn_w)
                    pup_w, pdn_w = pair

                    def _wdst(buf, parts=slice(None)):
                        b = buf[parts, half * W : half * W + W]
                        return _dc.replace(b, ap=[b.ap[0], [2 * W, 3], [1, W]])

                    def _wsrc(p, parts=slice(None)):
                        b = p[parts, 0:W]
                        return _dc.replace(b, ap=[b.ap[0], [1, 3], [1, W]])

                    nc.scalar.copy(_wdst(pup_w), _wsrc(pup))
                    nc.scalar.copy(_wdst(pdn_w), _wsrc(pdn))
                    if jb > 0:
                        # row 0 of pup = row 127 of the band above (shadow at partition 0)
                        nc.scalar.copy(
                            _wdst(pup_w, slice(0, 1)), _wsrc(sh127[j - 1], slice(0, 1))
                        )

                    # --- upd = 7*u + sum_k y_k  (PSUM accumulate) ---
                    u_in = u_band[:, 1 : W + 1]
                    upd = upd_pool.tile([128, W], F32, name="upd")
                    nc.tensor.matmul(upd[:], IDENT7[:], u_in, start=True, stop=False)
                    yE, yW = yEW[img]
                    nc.tensor.matmul(
                        upd[:], IDENT_L[:], yE[:, jb * W : (jb + 1) * W],
                        start=False, stop=False,
                    )
                    nc.tensor.matmul(
                        upd[:], IDENT_L[:], yW[:, jb * W : (jb + 1) * W],
                        start=False, stop=False,
                    )
                    upds[jb] = upd
                    if half == 0:
                        continue  # N/S, acc-close and finals happen at the odd band

                    # --- paired N/S customs over both bands (FD = 2*512) ---
                    def pair2(base_ap):
                        return _dc.replace(base_ap, ap=[base_ap.ap[0], [WP, 2], [1, W]])

                    def cont3(buf, off):
                        b = buf[:, off * 2 * W : off * 2 * W + W]
                        return _dc.replace(b, ap=[b.ap[0], [W, 2], [1, W]])

                    u_pair = pair2(u_cur[img][:, (jb - 1) * WP + 1 : (jb - 1) * WP + 1 + W])
                    yN = y_pool.tile([128, 2 * W], BF16, tag="yN", name="yN", bufs=3)
                    yS = y_pool.tile([128, 2 * W], BF16, tag="yS", name="yS", bufs=3)
                    # (src tile, col offset, direction k) for the four diagonals
                    ydiag = []
                    for k, (ptile, off) in {4: (pup_w, 2), 5: (pdn_w, 2),
                                            6: (pdn_w, 0), 7: (pup_w, 0)}.items():
                        yD = y_pool.tile([128, 2 * W], BF16, tag=f"yD{k}", name=f"yD{k}", bufs=3)
                        nc.vector._custom_dve(
                            PM_EW_OP,
                            out=_dc.replace(yD[:, :], ap=[yD.ap[0], [W, 2], [1, W]]),
                            in0=cont3(ptile, off), in1=u_pair,
                            s0=float(DIR_W[k] * factors[k, ch] / (PM_EW_L / 2)),
                            s1=float(DIR_W[k] * biases[k, ch] / (PM_EW_L / 2)),
                        )
                        ydiag.append(yD)
                    nc.vector._custom_dve(
                        PM_EW_OP, out=_dc.replace(yN[:, :], ap=[yN.ap[0], [W, 2], [1, W]]),
                        in0=cont3(pup_w, 1), in1=u_pair,
                        s0=float(factors[0, ch] / PM_EW_L),
                        s1=float(biases[0, ch] / PM_EW_L),
                    )
                    nc.vector._custom_dve(
                        PM_EW_OP, out=_dc.replace(yS[:, :], ap=[yS.ap[0], [W, 2], [1, W]]),
                        in0=cont3(pdn_w, 1), in1=u_pair,
                        s0=float(factors[1, ch] / PM_EW_L),
                        s1=float(biases[1, ch] / PM_EW_L),
                    )
                    for hh, jj in ((0, j - 1), (1, j)):
                        updx = upds[jj % BANDS_PER_IMG]
                        for yD in ydiag:
                            nc.tensor.matmul(
                                updx[:], IDENT_L2[:],
                                yD[:, hh * W : (hh + 1) * W], start=False, stop=False,
                            )
                        nc.tensor.matmul(
                            updx[:], IDENT_L[:],
                            yN[:, hh * W : (hh + 1) * W], start=False, stop=False,
                        )
                        nc.tensor.matmul(
                            updx[:], IDENT_L[:],
                            yS[:, hh * W : (hh + 1) * W], start=False, stop=True,
                        )

                    # --- u_{t+1} = DT * upd  (= u_t + DT * sum y), both bands ---
                    for jj in (j - 1, j):
                        jbx = jj % BANDS_PER_IMG
                        updx = upds[jbx]
                        if t < NUM_ITER - 1:
                            nc.scalar.activation(
                                uv(u_nxt, jj)[:, 1 : W + 1], updx[:],
                                mybir.ActivationFunctionType.Copy, scale=float(DT),
                            )
                        else:
                            so = io_pool.tile([128, W], F32, tag="stage_out", name="so")
                            nc.scalar.activation(
                                so[:], updx[:],
                                mybir.ActivationFunctionType.Copy, scale=float(DT),
                            )
                            nc.sync.dma_start(o_d[img, jbx * 128 : (jbx + 1) * 128, :], so[:])

                # refresh row-127 shadows for the next iteration; emitted after
                # every band's reads of the old shadow values so Tile orders
                # write-after-read correctly
                if t < NUM_ITER - 1:
                    for j in range(N_BANDS):
                        if j % BANDS_PER_IMG < BANDS_PER_IMG - 1:
                            nc.sync.dma_start(sh127[j][0:1, :], uv(u_nxt, j)[127:128, :])

    nc.finalize()
    return nc


def _install_ntff_hook():
    """The agent image's antenv lacks axon_hooks; recreate it so trace=True works."""
    import types

    try:
        from antenv.axon_hooks import get_axon_ntff_profile_hook  # noqa: F401

        return
    except ImportError:
        pass
    import antenv

    mod = types.ModuleType("antenv.axon_hooks")
    _state = {"hook": None}
    mod.set_axon_ntff_profile_hook = lambda h: _state.__setitem__("hook", h)
    mod.get_axon_ntff_profile_hook = lambda: _state["hook"]
    sys.modules["antenv.axon_hooks"] = mod
    antenv.axon_hooks = mod
    so_path = "/opt/axon/libaxon_pjrt.so"
    if os.path.exists(so_path):
        sys.path.insert(0, "/root/.axon_site")
        try:
            from trn_agent_boot.trn_boot import _ntff_profile_via_ctypes

            hook = _ntff_profile_via_ctypes(so_path)
            if hook is not None:
                mod.set_axon_ntff_profile_hook(hook)
        except Exception as e:
            print(f"ntff hook install failed: {e}")


_CACHE = {}


def _get_nc(biases, factors):
    key = (biases.tobytes(), factors.tobytes())
    if key not in _CACHE:
        _CACHE[key] = build_nc(biases, factors)
    return _CACHE[key]


def kernel(x, biases, factors, _trace=False):
    x = np.ascontiguousarray(np.asarray(x, np.float32))
    biases = np.asarray(biases, np.float32)
    factors = np.asarray(factors, np.float32)
    nc = _get_nc(biases, factors)
    if _trace:
        _install_ntff_hook()

    wmat = _weight_mats()
    per_core = B // N_CORES
    in_maps = [
        {
            "x": x[i * per_core : (i + 1) * per_core].reshape(IMGS, H, W),
            "wmat": wmat,
        }
        for i in range(N_CORES)
    ]
    res = run_bass_kernel_spmd(nc, in_maps, core_ids=list(range(N_CORES)), trace=_trace)
    out = np.concatenate(
        [res.results[i]["out"].reshape(per_core, C, H, W) for i in range(N_CORES)],
        axis=0,
    )
    if _trace:
        kernel.last_exec_time_ns = res.exec_time_ns
        kernel.last_results = res
    return out


# revision 34
# speedup vs baseline: 1.1470x; 1.1470x over previous
"""Perona-Malik anisotropic diffusion (option 2), 10 iterations, on 8 TRN2 NeuronCores.

Pure data parallel: each core takes 2 of the 16 batch images (= 6 channel-images of
512x512).  Per core, u is double-buffered in SBUF as 6 per-image tiles
[128 rows, 4 bands x 514 cols] bf16 (512 interior cols + 2 zero-pad cols giving
zero-padding semantics for horizontal shifts; 512 rows = exactly 4 x 128 partitions).

Division of labor per band per iteration:
  - TensorEngine produces row-shifted copies pup/pdn [128,514] in PSUM via shift-matrix
    matmuls (band-seam rows come from a one-hot matmul of the band below, and from a
    DMA-maintained partition-0 "shadow" of each band's row 127 for the band above,
    since compute engines cannot address partition 127 directly).
  - ScalarEngine stages pup/pdn to SBUF (ScalarE reads PSUM fast, VectorE reads SBUF
    fast) and applies the final u_{t+1} = DT * upd update (PSUM -> bf16 SBUF).
  - VectorEngine runs ONE fused custom DVE op per direction:
        y_k = (w f d + w b) * (1 - (f d)^2 / (2 kappa^2))^2,   d = shift_k(u) - u
    approximating w * nab / (1 + (nab/kappa)^2) (Taylor in z = (nab/kappa)^2 <= 0.09).
    E/W are batched across all 4 bands of an image in a single FD=2048 op (a no-imm2
    variant whose output scale 42.5 rides the accumulating matmul's lhsT).
  - TensorEngine sums the 8 directional fields plus 7*u into PSUM via (scaled-)identity
    matmuls; 1/DT = 7 is folded into the u term so the final update is a pure scale.

biases/factors are folded into the custom-op scalars at trace time (the kernel is
compiled inside kernel(), cached on the biases/factors bytes).  Measured end-to-end
max rel err vs the exact f32 reference: 3.7e-3 (bf16 state + Taylor approx), well
inside the 2e-2 gate.  HW exec time ~1.39 ms; VectorE (the bottleneck) is >97% busy
at its per-op streaming floor.
"""
import math
import os
import sys

import numpy as np

for _p in ("/root/.axon_site", "/root/.axon_site/_ro/trn_rl_repo", "/opt/trn_rl_repo"):
    if os.path.isdir(_p) and _p not in sys.path:
        sys.path.append(_p)

import concourse.bass as bass
import concourse.tile as tile
from concourse import bacc, mybir
from concourse.bass_utils import run_bass_kernel_spmd

# ---------------- problem constants (hardcoded; kernel.py is self-contained) ---
B, C, H, W = 16, 3, 512, 512
NUM_ITER = 10
DT = 1.0 / 7.0
KAPPA = 30.0
OFFSETS = [(-1, 0), (1, 0), (0, -1), (0, 1), (-1, 1), (1, 1), (1, -1), (-1, -1)]
DIR_W = [1.0, 1.0, 1.0, 1.0, 0.5, 0.5, 0.5, 0.5]

N_CORES = 8
IMGS = (B // N_CORES) * C          # 6 images per core
BANDS_PER_IMG = H // 128           # 4
N_BANDS = IMGS * BANDS_PER_IMG     # 24
WP = W + 2                         # padded width 514

BF16 = mybir.dt.bfloat16
F32 = mybir.dt.float32

# ---------------- custom DVE op: fused diffusion direction ---------------------
from concourse.dve_spec import Spec, Src0, Src1, One, sq, lower
from concourse.dve_ops import (
    OPS,
    DveOp,
    _SUB_OPCODE_FOR_NAME,
    _CUSTOM_DVE_ROW_BASE,
    C0,
    C1,
    C2,
)
from concourse.dve_uop import DveOpSpec


def _pm_ref(in0, in1, s0, s1, imm2):
    d = in0.astype(np.float32) - in1.astype(np.float32)
    m = d * s0
    nt = m + s1
    v = m * imm2
    g = 1.0 - v * v
    return nt * (g * g)


def _register_pm_op():
    name = "PM_DIFFUSE_ANT"
    if name in _SUB_OPCODE_FOR_NAME:
        return next(op for op in OPS if op.name == name)
    _d = Src0 - Src1
    _m = _d * C0
    _nt = _m + C1
    _v = _m * C2
    _g = One - sq(_v)
    spec = Spec(body=_nt * sq(_g), reference=_pm_ref)
    row = _CUSTOM_DVE_ROW_BASE + len(OPS)
    _SUB_OPCODE_FOR_NAME[name] = row
    shas = {}
    for ver in ("v3", "v4"):
        s = DveOpSpec(name=name, opcode=row, uops=lower(spec, ver=ver), rd1_en=True)
        shas[ver] = s.sha(ver)
    op = DveOp(name, spec, subdim=False, uops_sha=shas)
    OPS.append(op)
    return op


PM_OP = _register_pm_op()

# E/W variant: no imm2 slot available (3D in1), so the final scale L=42.5 is
# applied by the accumulating matmul (lhsT = 42.5*I, bf16-exact).
#   out = v*(1 - v^2)^2,  v = (in0-in1)*s0 + s1
# with s0 = w*f/L, s1 = w*b/L and L chosen ~= w*kappa*sqrt(2) so that
# v^2 ~= ((f d + b)/kappa)^2 / 2 (off by (42.4264/42.5)^2 = 0.35%, negligible).
PM_EW_L = 42.5


def _pm_ew_ref(in0, in1, s0, s1, imm2):
    v = (in0.astype(np.float32) - in1.astype(np.float32)) * s0 + s1
    g = 1.0 - v * v
    return v * (g * g)


def _register_pm_ew_op():
    name = "PM_DIFFUSE_EW_ANT"
    if name in _SUB_OPCODE_FOR_NAME:
        return next(op for op in OPS if op.name == name)
    _v = (Src0 - Src1) * C0 + C1
    _g = One - sq(_v)
    spec = Spec(body=_v * sq(_g), reference=_pm_ew_ref)
    row = _CUSTOM_DVE_ROW_BASE + len(OPS)
    _SUB_OPCODE_FOR_NAME[name] = row
    shas = {}
    for ver in ("v3", "v4"):
        sp = DveOpSpec(name=name, opcode=row, uops=lower(spec, ver=ver), rd1_en=True)
        shas[ver] = sp.sha(ver)
    op = DveOp(name, spec, subdim=False, uops_sha=shas)
    OPS.append(op)
    return op


PM_EW_OP = _register_pm_ew_op()


# ---------------- weight matrices for TensorE ---------------------------------
def _weight_mats():
    import ml_dtypes

    S_upT = np.zeros((128, 128), np.float32)   # out[m] = u[m-1]
    S_upT[np.arange(127), np.arange(1, 128)] = 1.0
    S_dnT = np.zeros((128, 128), np.float32)   # out[m] = u[m+1]
    S_dnT[np.arange(1, 128), np.arange(127)] = 1.0
    E_dnT = np.zeros((128, 128), np.float32)   # out[127] = next[0]
    E_dnT[0, 127] = 1.0
    I = np.eye(128, dtype=np.float32)
    I7 = 7.0 * np.eye(128, dtype=np.float32)   # folds 1/DT into the u term
    IL = PM_EW_L * np.eye(128, dtype=np.float32)   # w=1 custom-op output scale
    IL2 = (PM_EW_L / 2) * np.eye(128, dtype=np.float32)  # w=0.5 diagonals
    return np.stack([S_upT, S_dnT, E_dnT, I, I7, IL, IL2]).astype(ml_dtypes.bfloat16)


# ---------------- kernel build -------------------------------------------------
def build_nc(biases: np.ndarray, factors: np.ndarray):
    """Trace the full 10-iteration kernel; biases/factors folded as immediates."""
    biases = np.asarray(biases, np.float32)
    factors = np.asarray(factors, np.float32)

    nc = bacc.Bacc()
    x_d = nc.declare_dram_parameter("x", [IMGS, H, W], F32, isOutput=False)
    w_d = nc.declare_dram_parameter("wmat", [7, 128, 128], BF16, isOutput=False)
    o_d = nc.declare_dram_parameter("out", [IMGS, H, W], F32, isOutput=True)

    with tile.TileContext(nc) as tc:
        from contextlib import ExitStack

        with ExitStack() as ctx:
            upool = ctx.enter_context(tc.tile_pool(name="u", bufs=1))
            wpool = ctx.enter_context(tc.tile_pool(name="w", bufs=1))
            io_pool = ctx.enter_context(tc.tile_pool(name="io", bufs=4))
            y_pool = ctx.enter_context(tc.tile_pool(name="y", bufs=12))  # per-tag bufs below for big tiles
            pup_pool = ctx.enter_context(tc.tile_pool(name="pup", bufs=1, space="PSUM"))
            pdn_pool = ctx.enter_context(tc.tile_pool(name="pdn", bufs=1, space="PSUM"))
            upd_pool = ctx.enter_context(tc.tile_pool(name="upd", bufs=4, space="PSUM"))
            sh_pool = ctx.enter_context(tc.tile_pool(name="sh", bufs=1))
            ps_pool = ctx.enter_context(tc.tile_pool(name="ps", bufs=3))
            import dataclasses as _dc

            def _src_win3(row_ap):
                """[1, W]-row AP -> [1, 3, W] overlapping windows at col offsets 0,1,2."""
                return _dc.replace(row_ap, ap=[row_ap.ap[0], [1, 3], [1, W]])

            def _dst3(row_ap):
                """[1, 3W]-row AP -> [1, 3, W] contiguous split."""
                return _dc.replace(row_ap, ap=[row_ap.ap[0], [W, 3], [1, W]])

            # persistent tiles
            wt = [wpool.tile([128, 128], BF16, tag=f"w{i}", name=f"w{i}") for i in range(7)]
            S_UP, S_DN, E_DN, IDENT, IDENT7, IDENT_L, IDENT_L2 = wt
            uA = [upool.tile([128, BANDS_PER_IMG * WP], BF16, tag=f"uA{i}", name=f"uA{i}") for i in range(IMGS)]
            uB = [upool.tile([128, BANDS_PER_IMG * WP], BF16, tag=f"uB{i}", name=f"uB{i}") for i in range(IMGS)]

            def uv(ub, j):
                img, jb = divmod(j, BANDS_PER_IMG)
                return ub[img][:, jb * WP : (jb + 1) * WP]

            def img_win(ub, img, col, n=W):
                """[128, 4, n] view of image tile: 4 bands at column offset col."""
                base = ub[img][:, col : col + n]
                return _dc.replace(base, ap=[base.ap[0], [WP, BANDS_PER_IMG], [1, n]])

            def y4_split(y_ap):
                """[128, 4*W] tile -> [128, 4, W]."""
                return _dc.replace(y_ap, ap=[y_ap.ap[0], [W, BANDS_PER_IMG], [1, W]])
            # row 127 of each band mirrored at partition 0 (engines cannot read
            # partition 127 directly: partition starts must be quadrant-aligned)
            sh127 = [sh_pool.tile([1, WP], BF16, tag=f"sh{j}", name=f"sh{j}") for j in range(N_BANDS)]

            for i in range(7):
                nc.sync.dma_start(wt[i][:], w_d[i])

            # load input: DMA f32 -> staging, convert to bf16 interior; zero pads
            for j in range(N_BANDS):
                img, jb = divmod(j, BANDS_PER_IMG)
                st = io_pool.tile([128, W], F32, tag="stage_in")
                nc.sync.dma_start(st[:], x_d[img, jb * 128 : (jb + 1) * 128, :])
                for ub in (uA, uB):
                    v = uv(ub, j)
                    nc.gpsimd.memset(v[:, 0:1], 0.0)
                    nc.gpsimd.memset(v[:, WP - 1 : WP], 0.0)
                nc.scalar.copy(uv(uA, j)[:, 1 : W + 1], st[:])
                if jb < BANDS_PER_IMG - 1:
                    nc.sync.dma_start(sh127[j][0:1, :], uv(uA, j)[127:128, :])

            # per-direction constants
            # y = (w f d + w b) * (1 - (f d)^2/(2 kappa^2))^2
            # s0 = w*f[k,c], s1 = w*b[k,c], imm2 = 1/(w*kappa*sqrt(2))
            def consts(k, c):
                wgt = DIR_W[k]
                return (
                    float(wgt * factors[k, c]),
                    float(wgt * biases[k, c]),
                    float(1.0 / (wgt * KAPPA * math.sqrt(2.0))),
                )

            bufs = [uA, uB]
            for t in range(NUM_ITER):
                u_cur = bufs[t % 2]
                u_nxt = bufs[(t + 1) % 2]
                yEW = {}
                upds = {}
                pair = None
                for j in range(N_BANDS):
                    img, jb = divmod(j, BANDS_PER_IMG)
                    ch = img % C
                    if jb == 0:
                        # batched E/W custom ops over the whole image (FD = 4*512)
                        yE = y_pool.tile([128, BANDS_PER_IMG * W], BF16, tag="yE", name="yE", bufs=3)
                        yW = y_pool.tile([128, BANDS_PER_IMG * W], BF16, tag="yW", name="yW", bufs=3)
                        nc.vector._custom_dve(
                            PM_EW_OP, out=y4_split(yE[:, :]),
                            in0=img_win(u_cur, img, 2), in1=img_win(u_cur, img, 1),
                            s0=float(DIR_W[3] * factors[3, ch] / PM_EW_L),
                            s1=float(DIR_W[3] * biases[3, ch] / PM_EW_L),
                        )
                        nc.vector._custom_dve(
                            PM_EW_OP, out=y4_split(yW[:, :]),
                            in0=img_win(u_cur, img, 0), in1=img_win(u_cur, img, 1),
                            s0=float(DIR_W[2] * factors[2, ch] / PM_EW_L),
                            s1=float(DIR_W[2] * biases[2, ch] / PM_EW_L),
                        )
                        yEW[img] = (yE, yW)
                    # --- TensorE: row-shifted copies pup/pdn [128, 514] f32 ---
                    u_band = uv(u_cur, j)
                    pup = pup_pool.tile([128, WP], F32, name="pup")
                    pdn = pdn_pool.tile([128, WP], F32, name="pdn")
                    has_dn = jb < BANDS_PER_IMG - 1
                    for lo, hi in ((0, 512), (512, WP)):
                        nc.tensor.matmul(
                            pup[:, lo:hi], S_UP[:], u_band[:, lo:hi],
                            start=True, stop=True,
                        )
                        nc.tensor.matmul(
                            pdn[:, lo:hi], S_DN[:], u_band[:, lo:hi],
                            start=True, stop=not has_dn,
                        )
                        if has_dn:
                            nc.tensor.matmul(
                                pdn[:, lo:hi], E_DN[:], uv(u_cur, j + 1)[:, lo:hi],
                                start=False, stop=True,
                            )
                    # stage P into SBUF (ScalarE reads PSUM fast; DVE reads SBUF fast).
                    # Bands are staged in PAIRS (jb 0+1, 2+3) side by side so the
                    # N/S custom ops can process two bands in one FD=1024 stream.
                    half = jb % 2
                    if half == 0:
                        pup_s2 = ps_pool.tile([128, 2 * WP], F32, tag="pup_s", name="pup_s2")
                        pdn_s2 = ps_pool.tile([128, 2 * WP], F32, tag="pdn_s", name="pdn_s2")
                        pair = (pup_s2, pdn_s2)
                    pup_s2, pdn_s2 = pair
                    pup_s = pup_s2[:, half * WP : (half + 1) * WP]
                    pdn_s = pdn_s2[:, half * WP : (half + 1) * WP]
                    nc.scalar.copy(pup_s[:], pup[:])
                    nc.scalar.copy(pdn_s[:], pdn[:])
                    if jb > 0:
                        # row 0 of pup = row 127 of the band above (shadow at partition 0)
                        nc.scalar.copy(pup_s[0:1, :], sh127[j - 1][0:1, :])

                    # --- upd = 7*u + sum_k y_k  (PSUM accumulate) ---
                    u_in = u_band[:, 1 : W + 1]
                    upd = upd_pool.tile([128, W], F32, name="upd")
                    nc.tensor.matmul(upd[:], IDENT7[:], u_in, start=True, stop=False)
                    yE, yW = yEW[img]
                    nc.tensor.matmul(
                        upd[:], IDENT_L[:], yE[:, jb * W : (jb + 1) * W],
                        start=False, stop=False,
                    )
                    nc.tensor.matmul(
                        upd[:], IDENT_L[:], yW[:, jb * W : (jb + 1) * W],
                        start=False, stop=False,
                    )
                    upds[jb] = upd
                    if half == 0:
                        continue  # N/S, acc-close and finals happen at the odd band

                    # --- paired N/S customs over both bands (FD = 2*512) ---
                    def pair2(base_ap):
                        return _dc.replace(base_ap, ap=[base_ap.ap[0], [WP, 2], [1, W]])

                    u_pair = pair2(u_cur[img][:, (jb - 1) * WP + 1 : (jb - 1) * WP + 1 + W])
                    yN = y_pool.tile([128, 2 * W], BF16, tag="yN", name="yN", bufs=3)
                    yS = y_pool.tile([128, 2 * W], BF16, tag="yS", name="yS", bufs=3)
                    # (src tile, col offset, direction k) for the four diagonals
                    ydiag = []
                    for k, (ptile, off) in {4: (pup_s2, 2), 5: (pdn_s2, 2),
                                            6: (pdn_s2, 0), 7: (pup_s2, 0)}.items():
                        yD = y_pool.tile([128, 2 * W], BF16, tag=f"yD{k}", name=f"yD{k}", bufs=3)
                        nc.vector._custom_dve(
                            PM_EW_OP,
                            out=_dc.replace(yD[:, :], ap=[yD.ap[0], [W, 2], [1, W]]),
                            in0=pair2(ptile[:, off : off + W]), in1=u_pair,
                            s0=float(DIR_W[k] * factors[k, ch] / (PM_EW_L / 2)),
                            s1=float(DIR_W[k] * biases[k, ch] / (PM_EW_L / 2)),
                        )
                        ydiag.append(yD)
                    nc.vector._custom_dve(
                        PM_EW_OP, out=_dc.replace(yN[:, :], ap=[yN.ap[0], [W, 2], [1, W]]),
                        in0=pair2(pup_s2[:, 1 : 1 + W]), in1=u_pair,
                        s0=float(factors[0, ch] / PM_EW_L),
                        s1=float(biases[0, ch] / PM_EW_L),
                    )
                    nc.vector._custom_dve(
                        PM_EW_OP, out=_dc.replace(yS[:, :], ap=[yS.ap[0], [W, 2], [1, W]]),
                        in0=pair2(pdn_s2[:, 1 : 1 + W]), in1=u_pair,
                        s0=float(factors[1, ch] / PM_EW_L),
                        s1=float(biases[1, ch] / PM_EW_L),
                    )
                    for hh, jj in ((0, j - 1), (1, j)):
                        updx = upds[jj % BANDS_PER_IMG]
                        for yD in ydiag:
                            nc.tensor.matmul(
                                updx[:], IDENT_L2[:],
                                yD[:, hh * W : (hh + 1) * W], start=False, stop=False,
                            )
                        nc.tensor.matmul(
                            updx[:], IDENT_L[:],
                            yN[:, hh * W : (hh + 1) * W], start=False, stop=False,
                        )
                        nc.tensor.matmul(
                            updx[:], IDENT_L[:],
                            yS[:, hh * W : (hh + 1) * W], start=False, stop=True,
                        )

                    # --- u_{t+1} = DT * upd  (= u_t + DT * sum y), both bands ---
                    for jj in (j - 1, j):
                        jbx = jj % BANDS_PER_IMG
                        updx = upds[jbx]
                        if t < NUM_ITER - 1:
                            nc.scalar.activation(
                                uv(u_nxt, jj)[:, 1 : W + 1], updx[:],
                                mybir.ActivationFunctionType.Copy, scale=float(DT),
                            )
                        else:
                            so = io_pool.tile([128, W], F32, tag="stage_out", name="so")
                            nc.scalar.activation(
                                so[:], updx[:],
                                mybir.ActivationFunctionType.Copy, scale=float(DT),
                            )
                            nc.sync.dma_start(o_d[img, jbx * 128 : (jbx + 1) * 128, :], so[:])

                # refresh row-127 shadows for the next iteration; emitted after
                # every band's reads of the old shadow values so Tile orders
                # write-after-read correctly
                if t < NUM_ITER - 1:
                    for j in range(N_BANDS):
                        if j % BANDS_PER_IMG < BANDS_PER_IMG - 1:
                            nc.sync.dma_start(sh127[j][0:1, :], uv(u_nxt, j)[127:128, :])

    nc.finalize()
    return nc


def _install_ntff_hook():
    """The agent image's antenv lacks axon_hooks; recreate it so trace=True works."""
    import types

    try:
        from antenv.axon_hooks import get_axon_ntff_profile_hook  # noqa: F401

        return
    except ImportError:
        pass
    import antenv

    mod = types.ModuleType("antenv.axon_hooks")
    _state = {"hook": None}
    mod.set_axon_ntff_profile_hook = lambda h: _state.__setitem__("hook", h)
    mod.get_axon_ntff_profile_hook = lambda: _state["hook"]
    sys.modules["antenv.axon_hooks"] = mod
    antenv.axon_hooks = mod
    so_path = "/opt/axon/libaxon_pjrt.so"
    if os.path.exists(so_path):
        sys.path.insert(0, "/root/.axon_site")
        try:
            from trn_agent_boot.trn_boot import _ntff_profile_via_ctypes

            hook = _ntff_profile_via_ctypes(so_path)
            if hook is not None:
                mod.set_axon_ntff_profile_hook(hook)
        except Exception as e:
            print(f"ntff hook install failed: {e}")


_CACHE = {}


def _get_nc(biases, factors):
    key = (biases.tobytes(), factors.tobytes())
    if key not in _CACHE:
        _CACHE[key] = build_nc(biases, factors)
    return _CACHE[key]


def kernel(x, biases, factors, _trace=False):
    x = np.ascontiguousarray(np.asarray(x, np.float32))
    biases = np.asarray(biases, np.float32)
    factors = np.asarray(factors, np.float32)
    nc = _get_nc(biases, factors)
    if _trace:
        _install_ntff_hook()

    wmat = _weight_mats()
    per_core = B // N_CORES
    in_maps = [
        {
            "x": x[i * per_core : (i + 1) * per_core].reshape(IMGS, H, W),
            "wmat": wmat,
        }
        for i in range(N_CORES)
    ]
    res = run_bass_kernel_spmd(nc, in_maps, core_ids=list(range(N_CORES)), trace=_trace)
    out = np.concatenate(
        [res.results[i]["out"].reshape(per_core, C, H, W) for i in range(N_CORES)],
        axis=0,
    )
    if _trace:
        kernel.last_exec_time_ns = res.exec_time_ns
        kernel.last_results = res
    return out


# revision 35
# speedup vs baseline: 1.1471x; 1.0001x over previous
"""Perona-Malik anisotropic diffusion (option 2), 10 iterations, on 8 TRN2 NeuronCores.

Pure data parallel: each core takes 2 of the 16 batch images (= 6 channel-images of
512x512).  Per core, u is double-buffered in SBUF as 6 per-image tiles
[128 rows, 4 bands x 514 cols] bf16 (512 interior cols + 2 zero-pad cols giving
zero-padding semantics for horizontal shifts; 512 rows = exactly 4 x 128 partitions).

Division of labor per band per iteration:
  - TensorEngine produces row-shifted copies pup/pdn [128,514] in PSUM via shift-matrix
    matmuls (band-seam rows come from a one-hot matmul of the band below, and from a
    DMA-maintained partition-0 "shadow" of each band's row 127 for the band above,
    since compute engines cannot address partition 127 directly).
  - ScalarEngine stages pup/pdn to SBUF (ScalarE reads PSUM fast, VectorE reads SBUF
    fast) and applies the final u_{t+1} = DT * upd update (PSUM -> bf16 SBUF).
  - VectorEngine runs ONE fused custom DVE op per direction:
        y_k = (w f d + w b) * (1 - (f d)^2 / (2 kappa^2))^2,   d = shift_k(u) - u
    approximating w * nab / (1 + (nab/kappa)^2) (Taylor in z = (nab/kappa)^2 <= 0.09).
    E/W are batched across all 4 bands of an image in a single FD=2048 op (a no-imm2
    variant whose output scale 42.5 rides the accumulating matmul's lhsT).
  - TensorEngine sums the 8 directional fields plus 7*u into PSUM via (scaled-)identity
    matmuls; 1/DT = 7 is folded into the u term so the final update is a pure scale.

biases/factors are folded into the custom-op scalars at trace time (the kernel is
compiled inside kernel(), cached on the biases/factors bytes).  Measured end-to-end
max rel err vs the exact f32 reference: 3.7e-3 (bf16 state + Taylor approx), well
inside the 2e-2 gate.  HW exec time ~1.39 ms; VectorE (the bottleneck) is >97% busy
at its per-op streaming floor.
"""
import math
import os
import sys

import numpy as np

for _p in ("/root/.axon_site", "/root/.axon_site/_ro/trn_rl_repo", "/opt/trn_rl_repo"):
    if os.path.isdir(_p) and _p not in sys.path:
        sys.path.append(_p)

import concourse.bass as bass
import concourse.tile as tile
from concourse import bacc, mybir
from concourse.bass_utils import run_bass_kernel_spmd

# ---------------- problem constants (hardcoded; kernel.py is self-contained) ---
B, C, H, W = 16, 3, 512, 512
NUM_ITER = 10
DT = 1.0 / 7.0
KAPPA = 30.0
OFFSETS = [(-1, 0), (1, 0), (0, -1), (0, 1), (-1, 1), (1, 1), (1, -1), (-1, -1)]
DIR_W = [1.0, 1.0, 1.0, 1.0, 0.5, 0.5, 0.5, 0.5]

N_CORES = 8
IMGS = (B // N_CORES) * C          # 6 images per core
BANDS_PER_IMG = H // 128           # 4
N_BANDS = IMGS * BANDS_PER_IMG     # 24
WP = W + 2                         # padded width 514

BF16 = mybir.dt.bfloat16
F32 = mybir.dt.float32

# ---------------- custom DVE op: fused diffusion direction ---------------------
from concourse.dve_spec import Spec, Src0, Src1, One, sq, lower
from concourse.dve_ops import (
    OPS,
    DveOp,
    _SUB_OPCODE_FOR_NAME,
    _CUSTOM_DVE_ROW_BASE,
    C0,
    C1,
    C2,
)
from concourse.dve_uop import DveOpSpec


def _pm_ref(in0, in1, s0, s1, imm2):
    d = in0.astype(np.float32) - in1.astype(np.float32)
    m = d * s0
    nt = m + s1
    v = m * imm2
    g = 1.0 - v * v
    return nt * (g * g)


def _register_pm_op():
    name = "PM_DIFFUSE_ANT"
    if name in _SUB_OPCODE_FOR_NAME:
        return next(op for op in OPS if op.name == name)
    _d = Src0 - Src1
    _m = _d * C0
    _nt = _m + C1
    _v = _m * C2
    _g = One - sq(_v)
    spec = Spec(body=_nt * sq(_g), reference=_pm_ref)
    row = _CUSTOM_DVE_ROW_BASE + len(OPS)
    _SUB_OPCODE_FOR_NAME[name] = row
    shas = {}
    for ver in ("v3", "v4"):
        s = DveOpSpec(name=name, opcode=row, uops=lower(spec, ver=ver), rd1_en=True)
        shas[ver] = s.sha(ver)
    op = DveOp(name, spec, subdim=False, uops_sha=shas)
    OPS.append(op)
    return op


PM_OP = _register_pm_op()

# E/W variant: no imm2 slot available (3D in1), so the final scale L=42.5 is
# applied by the accumulating matmul (lhsT = 42.5*I, bf16-exact).
#   out = v*(1 - v^2)^2,  v = (in0-in1)*s0 + s1
# with s0 = w*f/L, s1 = w*b/L and L chosen ~= w*kappa*sqrt(2) so that
# v^2 ~= ((f d + b)/kappa)^2 / 2 (off by (42.4264/42.5)^2 = 0.35%, negligible).
PM_EW_L = 42.5


def _pm_ew_ref(in0, in1, s0, s1, imm2):
    v = (in0.astype(np.float32) - in1.astype(np.float32)) * s0 + s1
    g = 1.0 - v * v
    return v * (g * g)


def _register_pm_ew_op():
    name = "PM_DIFFUSE_EW_ANT"
    if name in _SUB_OPCODE_FOR_NAME:
        return next(op for op in OPS if op.name == name)
    _v = (Src0 - Src1) * C0 + C1
    _g = One - sq(_v)
    spec = Spec(body=_v * sq(_g), reference=_pm_ew_ref)
    row = _CUSTOM_DVE_ROW_BASE + len(OPS)
    _SUB_OPCODE_FOR_NAME[name] = row
    shas = {}
    for ver in ("v3", "v4"):
        sp = DveOpSpec(name=name, opcode=row, uops=lower(spec, ver=ver), rd1_en=True)
        shas[ver] = sp.sha(ver)
    op = DveOp(name, spec, subdim=False, uops_sha=shas)
    OPS.append(op)
    return op


PM_EW_OP = _register_pm_ew_op()


# ---------------- weight matrices for TensorE ---------------------------------
def _weight_mats():
    import ml_dtypes

    S_upT = np.zeros((128, 128), np.float32)   # out[m] = u[m-1]
    S_upT[np.arange(127), np.arange(1, 128)] = 1.0
    S_dnT = np.zeros((128, 128), np.float32)   # out[m] = u[m+1]
    S_dnT[np.arange(1, 128), np.arange(127)] = 1.0
    E_dnT = np.zeros((128, 128), np.float32)   # out[127] = next[0]
    E_dnT[0, 127] = 1.0
    I = np.eye(128, dtype=np.float32)
    I7 = 7.0 * np.eye(128, dtype=np.float32)   # folds 1/DT into the u term
    IL = PM_EW_L * np.eye(128, dtype=np.float32)   # w=1 custom-op output scale
    IL2 = (PM_EW_L / 2) * np.eye(128, dtype=np.float32)  # w=0.5 diagonals
    return np.stack([S_upT, S_dnT, E_dnT, I, I7, IL, IL2]).astype(ml_dtypes.bfloat16)


# ---------------- kernel build -------------------------------------------------
def build_nc(biases: np.ndarray, factors: np.ndarray):
    """Trace the full 10-iteration kernel; biases/factors folded as immediates."""
    biases = np.asarray(biases, np.float32)
    factors = np.asarray(factors, np.float32)

    nc = bacc.Bacc()
    x_d = nc.declare_dram_parameter("x", [IMGS, H, W], F32, isOutput=False)
    w_d = nc.declare_dram_parameter("wmat", [7, 128, 128], BF16, isOutput=False)
    o_d = nc.declare_dram_parameter("out", [IMGS, H, W], F32, isOutput=True)

    with tile.TileContext(nc) as tc:
        from contextlib import ExitStack

        with ExitStack() as ctx:
            upool = ctx.enter_context(tc.tile_pool(name="u", bufs=1))
            wpool = ctx.enter_context(tc.tile_pool(name="w", bufs=1))
            io_pool = ctx.enter_context(tc.tile_pool(name="io", bufs=4))
            y_pool = ctx.enter_context(tc.tile_pool(name="y", bufs=12))  # per-tag bufs below for big tiles
            pup_pool = ctx.enter_context(tc.tile_pool(name="pup", bufs=1, space="PSUM"))
            pdn_pool = ctx.enter_context(tc.tile_pool(name="pdn", bufs=1, space="PSUM"))
            upd_pool = ctx.enter_context(tc.tile_pool(name="upd", bufs=4, space="PSUM"))
            sh_pool = ctx.enter_context(tc.tile_pool(name="sh", bufs=1))
            ps_pool = ctx.enter_context(tc.tile_pool(name="ps", bufs=4))
            import dataclasses as _dc

            def _src_win3(row_ap):
                """[1, W]-row AP -> [1, 3, W] overlapping windows at col offsets 0,1,2."""
                return _dc.replace(row_ap, ap=[row_ap.ap[0], [1, 3], [1, W]])

            def _dst3(row_ap):
                """[1, 3W]-row AP -> [1, 3, W] contiguous split."""
                return _dc.replace(row_ap, ap=[row_ap.ap[0], [W, 3], [1, W]])

            # persistent tiles
            wt = [wpool.tile([128, 128], BF16, tag=f"w{i}", name=f"w{i}") for i in range(7)]
            S_UP, S_DN, E_DN, IDENT, IDENT7, IDENT_L, IDENT_L2 = wt
            uA = [upool.tile([128, BANDS_PER_IMG * WP], BF16, tag=f"uA{i}", name=f"uA{i}") for i in range(IMGS)]
            uB = [upool.tile([128, BANDS_PER_IMG * WP], BF16, tag=f"uB{i}", name=f"uB{i}") for i in range(IMGS)]

            def uv(ub, j):
                img, jb = divmod(j, BANDS_PER_IMG)
                return ub[img][:, jb * WP : (jb + 1) * WP]

            def img_win(ub, img, col, n=W):
                """[128, 4, n] view of image tile: 4 bands at column offset col."""
                base = ub[img][:, col : col + n]
                return _dc.replace(base, ap=[base.ap[0], [WP, BANDS_PER_IMG], [1, n]])

            def y4_split(y_ap):
                """[128, 4*W] tile -> [128, 4, W]."""
                return _dc.replace(y_ap, ap=[y_ap.ap[0], [W, BANDS_PER_IMG], [1, W]])
            # row 127 of each band mirrored at partition 0 (engines cannot read
            # partition 127 directly: partition starts must be quadrant-aligned)
            sh127 = [sh_pool.tile([1, WP], BF16, tag=f"sh{j}", name=f"sh{j}") for j in range(N_BANDS)]

            for i in range(7):
                nc.sync.dma_start(wt[i][:], w_d[i])

            # load input: DMA f32 -> staging, convert to bf16 interior; zero pads
            for j in range(N_BANDS):
                img, jb = divmod(j, BANDS_PER_IMG)
                st = io_pool.tile([128, W], F32, tag="stage_in")
                nc.sync.dma_start(st[:], x_d[img, jb * 128 : (jb + 1) * 128, :])
                for ub in (uA, uB):
                    v = uv(ub, j)
                    nc.gpsimd.memset(v[:, 0:1], 0.0)
                    nc.gpsimd.memset(v[:, WP - 1 : WP], 0.0)
                nc.scalar.copy(uv(uA, j)[:, 1 : W + 1], st[:])
                if jb < BANDS_PER_IMG - 1:
                    nc.sync.dma_start(sh127[j][0:1, :], uv(uA, j)[127:128, :])

            # per-direction constants
            # y = (w f d + w b) * (1 - (f d)^2/(2 kappa^2))^2
            # s0 = w*f[k,c], s1 = w*b[k,c], imm2 = 1/(w*kappa*sqrt(2))
            def consts(k, c):
                wgt = DIR_W[k]
                return (
                    float(wgt * factors[k, c]),
                    float(wgt * biases[k, c]),
                    float(1.0 / (wgt * KAPPA * math.sqrt(2.0))),
                )

            bufs = [uA, uB]
            for t in range(NUM_ITER):
                u_cur = bufs[t % 2]
                u_nxt = bufs[(t + 1) % 2]
                yEW = {}
                upds = {}
                pair = None
                for j in range(N_BANDS):
                    img, jb = divmod(j, BANDS_PER_IMG)
                    ch = img % C
                    if jb == 0:
                        # batched E/W custom ops over the whole image (FD = 4*512)
                        yE = y_pool.tile([128, BANDS_PER_IMG * W], BF16, tag="yE", name="yE", bufs=3)
                        yW = y_pool.tile([128, BANDS_PER_IMG * W], BF16, tag="yW", name="yW", bufs=3)
                        nc.vector._custom_dve(
                            PM_EW_OP, out=y4_split(yE[:, :]),
                            in0=img_win(u_cur, img, 2), in1=img_win(u_cur, img, 1),
                            s0=float(DIR_W[3] * factors[3, ch] / PM_EW_L),
                            s1=float(DIR_W[3] * biases[3, ch] / PM_EW_L),
                        )
                        nc.vector._custom_dve(
                            PM_EW_OP, out=y4_split(yW[:, :]),
                            in0=img_win(u_cur, img, 0), in1=img_win(u_cur, img, 1),
                            s0=float(DIR_W[2] * factors[2, ch] / PM_EW_L),
                            s1=float(DIR_W[2] * biases[2, ch] / PM_EW_L),
                        )
                        yEW[img] = (yE, yW)
                    # --- TensorE: row-shifted copies pup/pdn [128, 514] f32 ---
                    u_band = uv(u_cur, j)
                    pup = pup_pool.tile([128, WP], F32, name="pup")
                    pdn = pdn_pool.tile([128, WP], F32, name="pdn")
                    has_dn = jb < BANDS_PER_IMG - 1
                    for lo, hi in ((0, 512), (512, WP)):
                        nc.tensor.matmul(
                            pup[:, lo:hi], S_UP[:], u_band[:, lo:hi],
                            start=True, stop=True,
                        )
                        nc.tensor.matmul(
                            pdn[:, lo:hi], S_DN[:], u_band[:, lo:hi],
                            start=True, stop=not has_dn,
                        )
                        if has_dn:
                            nc.tensor.matmul(
                                pdn[:, lo:hi], E_DN[:], uv(u_cur, j + 1)[:, lo:hi],
                                start=False, stop=True,
                            )
                    # stage P into SBUF (ScalarE reads PSUM fast; DVE reads SBUF fast).
                    # Bands are staged in PAIRS (jb 0+1, 2+3) side by side so the
                    # N/S custom ops can process two bands in one FD=1024 stream.
                    half = jb % 2
                    if half == 0:
                        pup_s2 = ps_pool.tile([128, 2 * WP], F32, tag="pup_s", name="pup_s2")
                        pdn_s2 = ps_pool.tile([128, 2 * WP], F32, tag="pdn_s", name="pdn_s2")
                        pair = (pup_s2, pdn_s2)
                    pup_s2, pdn_s2 = pair
                    pup_s = pup_s2[:, half * WP : (half + 1) * WP]
                    pdn_s = pdn_s2[:, half * WP : (half + 1) * WP]
                    nc.scalar.copy(pup_s[:], pup[:])
                    nc.scalar.copy(pdn_s[:], pdn[:])
                    if jb > 0:
                        # row 0 of pup = row 127 of the band above (shadow at partition 0)
                        nc.scalar.copy(pup_s[0:1, :], sh127[j - 1][0:1, :])

                    # --- upd = 7*u + sum_k y_k  (PSUM accumulate) ---
                    u_in = u_band[:, 1 : W + 1]
                    upd = upd_pool.tile([128, W], F32, name="upd")
                    nc.tensor.matmul(upd[:], IDENT7[:], u_in, start=True, stop=False)
                    yE, yW = yEW[img]
                    nc.tensor.matmul(
                        upd[:], IDENT_L[:], yE[:, jb * W : (jb + 1) * W],
                        start=False, stop=False,
                    )
                    nc.tensor.matmul(
                        upd[:], IDENT_L[:], yW[:, jb * W : (jb + 1) * W],
                        start=False, stop=False,
                    )
                    upds[jb] = upd
                    if half == 0:
                        continue  # N/S, acc-close and finals happen at the odd band

                    # --- paired N/S customs over both bands (FD = 2*512) ---
                    def pair2(base_ap):
                        return _dc.replace(base_ap, ap=[base_ap.ap[0], [WP, 2], [1, W]])

                    u_pair = pair2(u_cur[img][:, (jb - 1) * WP + 1 : (jb - 1) * WP + 1 + W])
                    yN = y_pool.tile([128, 2 * W], BF16, tag="yN", name="yN", bufs=4)
                    yS = y_pool.tile([128, 2 * W], BF16, tag="yS", name="yS", bufs=4)
                    # (src tile, col offset, direction k) for the four diagonals
                    ydiag = []
                    for k, (ptile, off) in {4: (pup_s2, 2), 5: (pdn_s2, 2),
                                            6: (pdn_s2, 0), 7: (pup_s2, 0)}.items():
                        yD = y_pool.tile([128, 2 * W], BF16, tag=f"yD{k}", name=f"yD{k}", bufs=4)
                        nc.vector._custom_dve(
                            PM_EW_OP,
                            out=_dc.replace(yD[:, :], ap=[yD.ap[0], [W, 2], [1, W]]),
                            in0=pair2(ptile[:, off : off + W]), in1=u_pair,
                            s0=float(DIR_W[k] * factors[k, ch] / (PM_EW_L / 2)),
                            s1=float(DIR_W[k] * biases[k, ch] / (PM_EW_L / 2)),
                        )
                        ydiag.append(yD)
                    nc.vector._custom_dve(
                        PM_EW_OP, out=_dc.replace(yN[:, :], ap=[yN.ap[0], [W, 2], [1, W]]),
                        in0=pair2(pup_s2[:, 1 : 1 + W]), in1=u_pair,
                        s0=float(factors[0, ch] / PM_EW_L),
                        s1=float(biases[0, ch] / PM_EW_L),
                    )
                    nc.vector._custom_dve(
                        PM_EW_OP, out=_dc.replace(yS[:, :], ap=[yS.ap[0], [W, 2], [1, W]]),
                        in0=pair2(pdn_s2[:, 1 : 1 + W]), in1=u_pair,
                        s0=float(factors[1, ch] / PM_EW_L),
                        s1=float(biases[1, ch] / PM_EW_L),
                    )
                    for hh, jj in ((0, j - 1), (1, j)):
                        updx = upds[jj % BANDS_PER_IMG]
                        for yD in ydiag:
                            nc.tensor.matmul(
                                updx[:], IDENT_L2[:],
                                yD[:, hh * W : (hh + 1) * W], start=False, stop=False,
                            )
                        nc.tensor.matmul(
                            updx[:], IDENT_L[:],
                            yN[:, hh * W : (hh + 1) * W], start=False, stop=False,
                        )
                        nc.tensor.matmul(
                            updx[:], IDENT_L[:],
                            yS[:, hh * W : (hh + 1) * W], start=False, stop=True,
                        )

                    # --- u_{t+1} = DT * upd  (= u_t + DT * sum y), both bands ---
                    for jj in (j - 1, j):
                        jbx = jj % BANDS_PER_IMG
                        updx = upds[jbx]
                        if t < NUM_ITER - 1:
                            nc.scalar.activation(
                                uv(u_nxt, jj)[:, 1 : W + 1], updx[:],
                                mybir.ActivationFunctionType.Copy, scale=float(DT),
                            )
                        else:
                            so = io_pool.tile([128, W], F32, tag="stage_out", name="so")
                            nc.scalar.activation(
                                so[:], updx[:],
                                mybir.ActivationFunctionType.Copy, scale=float(DT),
                            )
                            nc.sync.dma_start(o_d[img, jbx * 128 : (jbx + 1) * 128, :], so[:])

                # refresh row-127 shadows for the next iteration; emitted after
                # every band's reads of the old shadow values so Tile orders
                # write-after-read correctly
                if t < NUM_ITER - 1:
                    for j in range(N_BANDS):
                        if j % BANDS_PER_IMG < BANDS_PER_IMG - 1:
                            nc.sync.dma_start(sh127[j][0:1, :], uv(u_nxt, j)[127:128, :])

    nc.finalize()
    return nc


def _install_ntff_hook():
    """The agent image's antenv lacks axon_hooks; recreate it so trace=True works."""
    import types

    try:
        from antenv.axon_hooks import get_axon_ntff_profile_hook  # noqa: F401

        return
    except ImportError:
        pass
    import antenv

    mod = types.ModuleType("antenv.axon_hooks")
    _state = {"hook": None}
    mod.set_axon_ntff_profile_hook = lambda h: _state.__setitem__("hook", h)
    mod.get_axon_ntff_profile_hook = lambda: _state["hook"]
    sys.modules["antenv.axon_hooks"] = mod
    antenv.axon_hooks = mod
    so_path = "/opt/axon/libaxon_pjrt.so"
    if os.path.exists(so_path):
        sys.path.insert(0, "/root/.axon_site")
        try:
            from trn_agent_boot.trn_boot import _ntff_profile_via_ctypes

            hook = _ntff_profile_via_ctypes(so_path)
            if hook is not None:
                mod.set_axon_ntff_profile_hook(hook)
        except Exception as e:
            print(f"ntff hook install failed: {e}")


_CACHE = {}


def _get_nc(biases, factors):
    key = (biases.tobytes(), factors.tobytes())
    if key not in _CACHE:
        _CACHE[key] = build_nc(biases, factors)
    return _CACHE[key]


def kernel(x, biases, factors, _trace=False):
    x = np.ascontiguousarray(np.asarray(x, np.float32))
    biases = np.asarray(biases, np.float32)
    factors = np.asarray(factors, np.float32)
    nc = _get_nc(biases, factors)
    if _trace:
        _install_ntff_hook()

    wmat = _weight_mats()
    per_core = B // N_CORES
    in_maps = [
        {
            "x": x[i * per_core : (i + 1) * per_core].reshape(IMGS, H, W),
            "wmat": wmat,
        }
        for i in range(N_CORES)
    ]
    res = run_bass_kernel_spmd(nc, in_maps, core_ids=list(range(N_CORES)), trace=_trace)
    out = np.concatenate(
        [res.results[i]["out"].reshape(per_core, C, H, W) for i in range(N_CORES)],
        axis=0,
    )
    if _trace:
        kernel.last_exec_time_ns = res.exec_time_ns
        kernel.last_results = res
    return out
